# revision 26
# baseline (speedup 1.0000x reference)
"""Trainium2 Bass kernel for nn_AttentionLayer (B=4, N=4096, D=128).

Computation (per reference):
    Q = h @ Wq + bq ; K = h @ Wk + bk ; V = h @ Wv + bv          [B, N, 128]
    scores = einsum("bnd,bmd->bnm", K, Q) / sqrt(128)            [B, N, N]
    attn = softmax(scores, axis=-1)
    out = einsum("bnm,bmd->bnd", attn, V)                        [B, N, N->128]

Sharding: 8 cores = 4 batches x 2 chunks of 2048 K-rows (output rows).
Fully data-parallel SPMD - no collectives. Each core receives its batch's
h rows PERMUTED so that its own K-chunk rows come first (softmax/PV reduce
over the m index, which is order-independent), and TRANSPOSED to [128, 4096]
so it DMAs straight into the on-chip hT layout. The output is returned in
[128 d, 2048 n] layout and un-transposed on the host.

Per-core kernel (hT resident after one 2MB load):
  kT   = Wk^T-proj of hT[:, :2048] + bk  (f32r)   [128d, 2048n]
  qT   = Wq^T-proj of hT + bq            (f32r)   [128d, 4096m]
  vN   = hT_tile^T @ Wv  (direct natural V tiles, bf16, NO bias --
         softmax rows sum to 1 so  attn @ (hWv + bv) = attn @ (hWv) + bv,
         bv is added per-partition in the epilogue)  [4096m, 128d]
  per (nh half of n, pair p of m-tiles):
      sT  = qT_mi^T @ kT_nh            (PSUM, f32r)       [128m, 1024n]
      e   = exp(sT / sqrt(128))        (ACT -> fp8e4)     [128m, 2, 1024n]
      oT += vN_mi^T @ e[:, j, :]       (PSUM accum, bf16 x fp8)
      den += ones8^T @ e (DoubleRow)   (PSUM accum, fp8 pair contraction)
  out_nh = oT * (1/den) + bv           (DVE)  -> DMA out in [d, n] layout

ACT (exp) is the bottleneck engine (~66us busy; 64 x [128,1024] EXPs).
PE ~62us, DVE ~18us. Projection groups beyond g0 are emitted inside the
early main-loop pairs so the first exp starts ~4us into the pass; each
nh epilogue is deferred until after the next nh's first pair so the ACT
exp stream never pauses at the nh boundary.
"""

import math
from contextlib import ExitStack

import numpy as np

import concourse.bass as bass
import concourse.mybir as mybir
import concourse.tile as tile
from concourse.bass_utils import run_bass_kernel_spmd
from concourse.masks import make_identity
from concourse.tile import ScopedClock

F32 = mybir.dt.float32
F32R = mybir.dt.float32r
BF16 = mybir.dt.bfloat16
F8 = mybir.dt.float8e4
DR = mybir.MatmulPerfMode.DoubleRow

B, N, D = 4, 4096, 128
NCORES = 8
CHUNK = N * B // NCORES  # 2048 output rows per core
NW = 512  # n processed per PSUM-resident accumulation group
SCALE = 1.0 / math.sqrt(D)


def _patched_drain_and_barrier(self, tick_clock, wait_clock):
    # This walrus build rejects multiple sync waits on the Drain CTRL
    # instruction. Carry the waits on preceding SP nops (same engine =>
    # program order) and leave the drain nearly bare.
    nc = self.nc
    carrier = nc.sync.nop(nofuse=True, hint="drain_waits")
    wait_clock.add_sem_waits(carrier.ins, ScopedClock({None: tick_clock.global_clock}))
    si = carrier.ins.sync_info
    waits = list(si.on_wait) if si is not None else []
    if len(waits) > 1:
        by_name = {}
        for h in self.sems.allocated().values():
            by_name[getattr(h, "name", None) or str(h)] = h
        si.on_wait = [waits[0]]
        for w in waits[1:]:
            n = nc.sync.nop(nofuse=True, hint="drain_waits2")
            n.wait_op(by_name[w.ant_name], w.wait_value, "sem-ge")
    nc.sync.drain()
    nc.all_engine_barrier()
    assert self.sems is not None
    popped = nc._tile_sem_poison_stack.pop()
    assert popped is self._sem_poison
    nc.clear_and_free_semaphores(list(self.sems.allocated().values()))
    nc.all_engine_barrier()


def ts(i, sz):
    return slice(i * sz, (i + 1) * sz)


def _split_excess_waits(nc, maxw=1):
    # This walrus build allows at most ~1 sync wait per lowered instruction.
    # Hoist excess waits onto preceding same-engine NoOps.
    cnt = 0
    for f in nc.m.functions:
        for bb in f.blocks:
            out = []
            for inst in bb.instructions:
                si = inst.sync_info
                waits = list(si.on_wait) if si is not None else []
                if len(waits) > maxw:
                    for w in waits[: len(waits) - maxw]:
                        nop = mybir.InstNoOp(
                            name=f"{inst.name}-hw{cnt}",
                            engine=inst.engine,
                            ins=[],
                            outs=[],
                            sync_info=mybir.SyncInfo(on_wait=[w], on_update=[]),
                        )
                        out.append(nop)
                        cnt += 1
                    si.on_wait = waits[len(waits) - maxw :]
                out.append(inst)
            bb.instructions = out
    return cnt


def build_nc(n=N, chunk=CHUNK, nw=NW, split_waits=True, repeat=1, unroll=1):
    M_TILES = n // 128
    NH = chunk // nw  # 4
    GRP = 8  # m-tiles per projection/psum group
    GW = GRP * 128
    PAIRS = M_TILES // 2  # m-tile pairs per nh
    tile.TileContext._drain_and_barrier = _patched_drain_and_barrier
    nc = bass.Bass("TRN2", target_bir_lowering=False, debug=False, num_devices=NCORES)

    h_d = nc.dram_tensor("h", [D, n], F32R, kind="ExternalInput")  # pre-transposed
    w_d = nc.dram_tensor("wqkv", [3, D, D], F32, kind="ExternalInput")
    b_d = nc.dram_tensor("bqkv", [3, D], F32, kind="ExternalInput")
    out_d = nc.dram_tensor("out", [D, chunk], F32R, kind="ExternalOutput")

    with tile.TileContext(nc) as tc, ExitStack() as ctx:
        consts = ctx.enter_context(tc.tile_pool(name="consts", bufs=1))
        big = ctx.enter_context(tc.tile_pool(name="big", bufs=1))
        expp = ctx.enter_context(tc.tile_pool(name="expp", bufs=4))
        denp = ctx.enter_context(tc.tile_pool(name="denp", bufs=2))
        outn = ctx.enter_context(tc.tile_pool(name="outn", bufs=2))
        ps_s = ctx.enter_context(tc.tile_pool(name="ps_s", bufs=2, space="PSUM"))
        ps_p = ctx.enter_context(tc.tile_pool(name="ps_p", bufs=1, space="PSUM"))
        ps_o = ctx.enter_context(tc.tile_pool(name="ps_o", bufs=1, space="PSUM"))
        ps_d = ctx.enter_context(tc.tile_pool(name="ps_d", bufs=1, space="PSUM"))

        # ---- constants (3 DMAs + DVE casts; once, outside the repeat loop) --
        w_s = consts.tile([D, 3, D], F32, tag="w_s")
        b_s = consts.tile([D, 3], F32, tag="b_s")
        nc.sync.dma_start(out=w_s, in_=w_d.ap().rearrange("w c d -> c w d"))
        nc.sync.dma_start(out=b_s, in_=b_d.ap().rearrange("w d -> d w"))
        wq_r = consts.tile([D, D], F32R, tag="wq_r")
        wk_r = consts.tile([D, D], F32R, tag="wk_r")
        wv_r = consts.tile([D, D], F32R, tag="wv_r")
        nc.vector.tensor_copy(out=wq_r, in_=w_s[:, 0, :])
        nc.vector.tensor_copy(out=wk_r, in_=w_s[:, 1, :])
        nc.vector.tensor_copy(out=wv_r, in_=w_s[:, 2, :])
        bq_s, bk_s, bv_s = b_s[:, 0:1], b_s[:, 1:2], b_s[:, 2:3]
        ones_f = consts.tile([128, 256], F32, tag="ones_f")
        ones8 = consts.tile([128, 2, 128], F8, tag="ones8")
        nc.vector.memset(ones_f, 1.0)
        nc.vector.tensor_copy(out=ones8, in_=ones_f.rearrange("p (a b) -> p a b", a=2))

        # ---- load hT (once; outside the repeat loop) as 4 independent
        # tiles so the first projections start after ~1/4 of the transfer --
        hTt = [
            big.tile([128, GW], F32R, name=f"hT{g}", tag=f"hT{g}")
            for g in range(M_TILES // GRP)
        ]
        for g, t in enumerate(hTt):
            nc.sync.dma_start(out=t, in_=h_d.ap()[:, ts(g, GW)])

        qT = big.tile([128, n], F32R, tag="qT")
        kT = big.tile([128, chunk], F32R, tag="kT")
        vN = big.tile([128, M_TILES // 2, 2, 128], F8, tag="vN")

        def emit_kt(g):
            p_t = ps_p.tile([128, GW], F32, tag="pp")
            for j in range(GW // 512):
                nc.tensor.matmul(p_t[:, ts(j, 512)], wk_r, hTt[g][:, ts(j, 512)])
            nc.vector.tensor_scalar_add(
                out=kT[:, ts(g, GW)], in0=p_t[:, :GW], scalar1=bk_s
            )

        def emit_qt(g):
            p_t = ps_p.tile([128, GW], F32, tag="pp")
            for j in range(GW // 512):
                nc.tensor.matmul(p_t[:, ts(j, 512)], wq_r, hTt[g][:, ts(j, 512)])
            nc.vector.tensor_scalar_add(
                out=qT[:, ts(g, GW)], in0=p_t[:, :GW], scalar1=bq_s
            )

        def emit_vn(g):
            t_ps = ps_p.tile([128, GW], F32, tag="pp")
            for k in range(GRP):
                nc.tensor.matmul(t_ps[:, ts(k, 128)], hTt[g][:, ts(k, 128)], wv_r)
            nc.vector.tensor_copy(
                out=vN[:, ts(g, GRP // 2), :, :],
                in_=t_ps.rearrange("p (t j d) -> p t j d", j=2, d=128),
            )

        def body():
            # head: only group 0 of each projection before the main loop;
            # the rest is staged into the early pairs below.
            emit_kt(0)
            emit_qt(0)
            emit_vn(0)

            # staged projection work, keyed by (nh, pair): emitted into the
            # PE stream right after that pair's matmuls. Group g of qT/vN is
            # consumed starting at pair 4*g of EVERY nh (each nh sweeps all
            # m-tiles), so everything beyond g0 must land within nh 0.
            staged = {
                (0, 0): lambda: emit_qt(1),
                (0, 2): lambda: emit_vn(1),
                (0, 4): lambda: emit_qt(2),
                (0, 6): lambda: emit_vn(2),
                (0, 8): lambda: emit_qt(3),
                (0, 10): lambda: emit_vn(3),
                (1, 0): lambda: emit_kt(1),
            }

            out_r = out_d.ap()
            pending_epi = [None]

            def flush_epi():
                if pending_epi[0] is not None:
                    pending_epi[0]()
                    pending_epi[0] = None

            for nh in range(NH):
                o_t = ps_o.tile([128, nw], F32, tag="o")
                d_t = ps_d.tile([128, nw], F32, tag="d")
                kslice = kT[:, nh * nw : (nh + 1) * nw]

                def emit_scores(p, nh=nh, kslice=kslice):
                    # both m-tiles of pair p, same 512 n-cols, one psum tile
                    s_t = ps_s.tile([128, 2, nw], F32, tag="s")
                    for j in range(2):
                        nc.tensor.matmul(
                            s_t[:, j, :], qT[:, ts(2 * p + j, 128)], kslice
                        )
                    return s_t

                s_tiles = {0: emit_scores(0), 1: emit_scores(1)}
                for p in range(PAIRS):
                    e_pr = expp.tile([128, 2, nw], F8, tag="e")
                    s_t = s_tiles.pop(p)
                    nc.scalar.activation(
                        out=e_pr.rearrange("p a b -> p (a b)"),
                        in_=s_t.rearrange("p a b -> p (a b)"),
                        func=mybir.ActivationFunctionType.Exp,
                        scale=SCALE,
                    )
                    # PE order: s(p+2), PV(p), den(p) -- one 1024-wide exp per
                    # pair feeds one DoubleRow PV and one DoubleRow den.
                    if p + 2 < PAIRS:
                        s_tiles[p + 2] = emit_scores(p + 2)
                    if p == 0:
                        # previous nh's epilogue. Emitted after this nh's first
                        # exp (so the ACT stream crosses the boundary without a
                        # gap) but BEFORE this nh's first PV/den write o_t/d_t:
                        # the hazard tracker follows emission order, so the
                        # epilogue's o_t/d_t reads must precede the overwrite.
                        flush_epi()
                    nc.tensor.matmul(
                        o_t,
                        vN[:, p, :, :],
                        e_pr,
                        start=p == 0,
                        stop=p == PAIRS - 1,
                        perf_mode=DR,
                        skip_group_check=True,
                    )
                    nc.tensor.matmul(
                        d_t,
                        ones8,
                        e_pr,
                        start=p == 0,
                        stop=p == PAIRS - 1,
                        perf_mode=DR,
                        skip_group_check=True,
                    )
                    staged.pop((nh, p), lambda: None)()

                def epilogue(nh=nh, o_t=o_t, d_t=d_t):
                    # out = o_t/den + bv, in [d, n] layout (bv per-partition)
                    rden = denp.tile([128, nw], F32, tag="rden")
                    o_f = outn.tile([128, 2, nw], F32R, tag="o_f")
                    nc.vector.reciprocal(out=rden, in_=d_t)
                    nc.vector.tensor_mul(out=o_f[:, 0, :], in0=o_t, in1=rden)
                    nc.vector.tensor_scalar_add(
                        out=o_f[:, 1, :], in0=o_f[:, 0, :], scalar1=bv_s
                    )
                    nc.sync.dma_start(
                        out=out_r[:, nh * nw : (nh + 1) * nw], in_=o_f[:, 1, :]
                    )

                pending_epi[0] = epilogue

            flush_epi()

        if unroll > 1:
            for _ in range(unroll):
                body()
        elif repeat > 1:
            with tc.For_i(0, repeat, 1):
                body()
        else:
            body()

    if split_waits:
        _split_excess_waits(nc)
    return nc


def make_in_maps(h_a, Wq, bq, Wk, bk, Wv, bv):
    """Per-core input dicts: permuted+transposed h plus stacked weights."""
    h_a = np.ascontiguousarray(h_a, dtype=np.float32)
    consts = {
        "wqkv": np.ascontiguousarray(np.stack([Wq, Wk, Wv]), np.float32),
        "bqkv": np.ascontiguousarray(np.stack([bq, bk, bv]), np.float32),
    }
    in_maps = []
    for core in range(NCORES):
        b, half = divmod(core, 2)
        n0 = half * CHUNK
        # chunk rows first, the rest after (order of the tail is irrelevant)
        perm = np.concatenate(
            [h_a[b, n0 : n0 + CHUNK], h_a[b, :n0], h_a[b, n0 + CHUNK :]], axis=0
        )
        in_maps.append({"h": np.ascontiguousarray(perm.T), **consts})
    return in_maps


_NC_CACHE = None
_LAST_RESULTS = None
TRACE = False
REPEAT = 1


def kernel(h_a, Wq, bq, Wk, bk, Wv, bv):
    global _NC_CACHE, _LAST_RESULTS
    if _NC_CACHE is None:
        _NC_CACHE = build_nc(repeat=REPEAT)
    nc = _NC_CACHE

    in_maps = make_in_maps(h_a, Wq, bq, Wk, bk, Wv, bv)
    res = run_bass_kernel_spmd(
        nc, in_maps, core_ids=list(range(NCORES)), trace=TRACE
    )
    _LAST_RESULTS = res

    out = np.empty((B, N, D), np.float32)
    for core in range(NCORES):
        b, half = divmod(core, 2)
        n0 = half * CHUNK
        out[b, n0 : n0 + CHUNK] = res.results[core]["out"].T
    return out


# revision 28
# speedup vs baseline: 1.0017x; 1.0017x over previous
"""Trainium2 Bass kernel for nn_AttentionLayer (B=4, N=4096, D=128).

Computation (per reference):
    Q = h @ Wq + bq ; K = h @ Wk + bk ; V = h @ Wv + bv          [B, N, 128]
    scores = einsum("bnd,bmd->bnm", K, Q) / sqrt(128)            [B, N, N]
    attn = softmax(scores, axis=-1)
    out = einsum("bnm,bmd->bnd", attn, V)                        [B, N, N->128]

Sharding: 8 cores = 4 batches x 2 chunks of 2048 K-rows (output rows).
Fully data-parallel SPMD - no collectives. Each core receives its batch's
h rows PERMUTED so that its own K-chunk rows come first (softmax/PV reduce
over the m index, which is order-independent), and TRANSPOSED to [128, 4096]
so it DMAs straight into the on-chip hT layout. The output is returned in
[128 d, 2048 n] layout and un-transposed on the host.

Per-core kernel (hT resident after one 2MB load):
  kT   = Wk^T-proj of hT[:, :2048] + bk  (f32r)   [128d, 2048n]
  qT   = Wq^T-proj of hT + bq            (f32r)   [128d, 4096m]
  vN   = hT_tile^T @ Wv  (direct natural V tiles, fp8e4, NO bias --
         softmax rows sum to 1 so  attn @ (hWv + bv) = attn @ (hWv) + bv,
         bv is added per-partition in the epilogue)  [4096m, 128d]
  per (nh quarter of n, pair p of m-tiles; one [128, 2, 512] psum score
  tile holds BOTH m-tiles of the pair for 512 n-cols):
      sT  = qT_mi^T @ kT_nh            (PSUM, f32r)       [128m, 2, 512n]
      e   = exp(sT / sqrt(128))        (one 1024-wide ACT -> fp8e4)
      oT += vN_pair^T @ e (DoubleRow)  (PSUM accum, fp8 pair contraction)
      den += ones8^T @ e  (DoubleRow)  (PSUM accum)
  out_nh = oT * (1/den) + bv           (DVE)  -> DMA out in [d, n] layout

ACT (exp) is the bottleneck engine (~66us busy; 64 x [128,1024] EXPs at
1 elem/lane/cycle @ 1.2 GHz -- the attention-math floor). PE ~53us,
DVE ~19us.  PSUM: 2x2-bank double-buffered score tiles + 2-bank
projection staging + 1-bank o/den accumulators = 8 banks.  Projection
groups beyond g0 are staged into the early main-loop pairs so the first
exp starts ~4us into the pass; each nh epilogue is deferred until after
the next nh's first exps (but before its first PV/den, which would
overwrite o/den) so the ACT exp stream never pauses at nh boundaries.
"""

import math
from contextlib import ExitStack

import numpy as np

import concourse.bass as bass
import concourse.mybir as mybir
import concourse.tile as tile
from concourse.bass_utils import run_bass_kernel_spmd
from concourse.masks import make_identity
from concourse.tile import ScopedClock

F32 = mybir.dt.float32
F32R = mybir.dt.float32r
BF16 = mybir.dt.bfloat16
F8 = mybir.dt.float8e4
DR = mybir.MatmulPerfMode.DoubleRow

B, N, D = 4, 4096, 128
NCORES = 8
CHUNK = N * B // NCORES  # 2048 output rows per core
NW = 512  # n processed per PSUM-resident accumulation group
SCALE = 1.0 / math.sqrt(D)


def _patched_drain_and_barrier(self, tick_clock, wait_clock):
    # This walrus build rejects multiple sync waits on the Drain CTRL
    # instruction. Carry the waits on preceding SP nops (same engine =>
    # program order) and leave the drain nearly bare.
    nc = self.nc
    carrier = nc.sync.nop(nofuse=True, hint="drain_waits")
    wait_clock.add_sem_waits(carrier.ins, ScopedClock({None: tick_clock.global_clock}))
    si = carrier.ins.sync_info
    waits = list(si.on_wait) if si is not None else []
    if len(waits) > 1:
        by_name = {}
        for h in self.sems.allocated().values():
            by_name[getattr(h, "name", None) or str(h)] = h
        si.on_wait = [waits[0]]
        for w in waits[1:]:
            n = nc.sync.nop(nofuse=True, hint="drain_waits2")
            n.wait_op(by_name[w.ant_name], w.wait_value, "sem-ge")
    nc.sync.drain()
    nc.all_engine_barrier()
    assert self.sems is not None
    popped = nc._tile_sem_poison_stack.pop()
    assert popped is self._sem_poison
    nc.clear_and_free_semaphores(list(self.sems.allocated().values()))
    nc.all_engine_barrier()


def ts(i, sz):
    return slice(i * sz, (i + 1) * sz)


def _split_excess_waits(nc, maxw=1):
    # This walrus build allows at most ~1 sync wait per lowered instruction.
    # Hoist excess waits onto preceding same-engine NoOps.
    cnt = 0
    for f in nc.m.functions:
        for bb in f.blocks:
            out = []
            for inst in bb.instructions:
                si = inst.sync_info
                waits = list(si.on_wait) if si is not None else []
                if len(waits) > maxw:
                    for w in waits[: len(waits) - maxw]:
                        nop = mybir.InstNoOp(
                            name=f"{inst.name}-hw{cnt}",
                            engine=inst.engine,
                            ins=[],
                            outs=[],
                            sync_info=mybir.SyncInfo(on_wait=[w], on_update=[]),
                        )
                        out.append(nop)
                        cnt += 1
                    si.on_wait = waits[len(waits) - maxw :]
                out.append(inst)
            bb.instructions = out
    return cnt


def build_nc(n=N, chunk=CHUNK, nw=NW, split_waits=True, repeat=1, unroll=1):
    M_TILES = n // 128
    NH = chunk // nw  # 4
    GRP = 8  # m-tiles per projection/psum group
    GW = GRP * 128
    PAIRS = M_TILES // 2  # m-tile pairs per nh
    tile.TileContext._drain_and_barrier = _patched_drain_and_barrier
    nc = bass.Bass("TRN2", target_bir_lowering=False, debug=False, num_devices=NCORES)

    h_d = nc.dram_tensor("h", [D, n], F32R, kind="ExternalInput")  # pre-transposed
    w_d = nc.dram_tensor("wqkv", [3, D, D], F32, kind="ExternalInput")
    b_d = nc.dram_tensor("bqkv", [3, D], F32, kind="ExternalInput")
    out_d = nc.dram_tensor("out", [D, chunk], F32R, kind="ExternalOutput")

    with tile.TileContext(nc) as tc, ExitStack() as ctx:
        consts = ctx.enter_context(tc.tile_pool(name="consts", bufs=1))
        big = ctx.enter_context(tc.tile_pool(name="big", bufs=1))
        expp = ctx.enter_context(tc.tile_pool(name="expp", bufs=3))
        denp = ctx.enter_context(tc.tile_pool(name="denp", bufs=2))
        outn = ctx.enter_context(tc.tile_pool(name="outn", bufs=2))
        ps_s = ctx.enter_context(tc.tile_pool(name="ps_s", bufs=2, space="PSUM"))
        ps_p = ctx.enter_context(tc.tile_pool(name="ps_p", bufs=1, space="PSUM"))
        ps_o = ctx.enter_context(tc.tile_pool(name="ps_o", bufs=1, space="PSUM"))
        ps_d = ctx.enter_context(tc.tile_pool(name="ps_d", bufs=1, space="PSUM"))

        # ---- constants (3 DMAs + DVE casts; once, outside the repeat loop) --
        w_s = consts.tile([D, 3, D], F32, tag="w_s")
        b_s = consts.tile([D, 3], F32, tag="b_s")
        nc.sync.dma_start(out=w_s, in_=w_d.ap().rearrange("w c d -> c w d"))
        nc.sync.dma_start(out=b_s, in_=b_d.ap().rearrange("w d -> d w"))
        wq_r = consts.tile([D, D], F32R, tag="wq_r")
        wk_r = consts.tile([D, D], F32R, tag="wk_r")
        wv_r = consts.tile([D, D], F32R, tag="wv_r")
        nc.vector.tensor_copy(out=wq_r, in_=w_s[:, 0, :])
        nc.vector.tensor_copy(out=wk_r, in_=w_s[:, 1, :])
        nc.vector.tensor_copy(out=wv_r, in_=w_s[:, 2, :])
        bq_s, bk_s, bv_s = b_s[:, 0:1], b_s[:, 1:2], b_s[:, 2:3]
        ones_f = consts.tile([128, 256], F32, tag="ones_f")
        ones8 = consts.tile([128, 2, 128], F8, tag="ones8")
        nc.vector.memset(ones_f, 1.0)
        nc.vector.tensor_copy(out=ones8, in_=ones_f.rearrange("p (a b) -> p a b", a=2))

        # ---- load hT (once; outside the repeat loop) as 4 independent
        # tiles so the first projections start after ~1/4 of the transfer --
        hTt = [
            big.tile([128, GW], F32R, name=f"hT{g}", tag=f"hT{g}")
            for g in range(M_TILES // GRP)
        ]
        for g, t in enumerate(hTt):
            nc.sync.dma_start(out=t, in_=h_d.ap()[:, ts(g, GW)])

        qT = big.tile([128, n], F32R, tag="qT")
        kT = big.tile([128, chunk], F32R, tag="kT")
        vN = big.tile([128, M_TILES // 2, 2, 128], F8, tag="vN")

        def emit_kt(g):
            p_t = ps_p.tile([128, GW], F32, tag="pp")
            for j in range(GW // 512):
                nc.tensor.matmul(p_t[:, ts(j, 512)], wk_r, hTt[g][:, ts(j, 512)])
            nc.vector.tensor_scalar_add(
                out=kT[:, ts(g, GW)], in0=p_t[:, :GW], scalar1=bk_s
            )

        def emit_qt(g):
            p_t = ps_p.tile([128, GW], F32, tag="pp")
            for j in range(GW // 512):
                nc.tensor.matmul(p_t[:, ts(j, 512)], wq_r, hTt[g][:, ts(j, 512)])
            nc.vector.tensor_scalar_add(
                out=qT[:, ts(g, GW)], in0=p_t[:, :GW], scalar1=bq_s
            )

        def emit_vn(g):
            t_ps = ps_p.tile([128, GW], F32, tag="pp")
            for k in range(GRP):
                nc.tensor.matmul(t_ps[:, ts(k, 128)], hTt[g][:, ts(k, 128)], wv_r)
            nc.vector.tensor_copy(
                out=vN[:, ts(g, GRP // 2), :, :],
                in_=t_ps.rearrange("p (t j d) -> p t j d", j=2, d=128),
            )

        def body():
            # head: only group 0 of each projection before the main loop;
            # the rest is staged into the early pairs below.
            emit_kt(0)
            emit_qt(0)
            emit_vn(0)

            # staged projection work, keyed by (nh, pair): emitted into the
            # PE stream right after that pair's matmuls. Group g of qT/vN is
            # consumed starting at pair 4*g of EVERY nh (each nh sweeps all
            # m-tiles), so everything beyond g0 must land within nh 0.
            staged = {
                (0, 0): lambda: emit_qt(1),
                (0, 2): lambda: emit_vn(1),
                (0, 4): lambda: emit_qt(2),
                (0, 6): lambda: emit_vn(2),
                (0, 8): lambda: emit_qt(3),
                (0, 10): lambda: emit_vn(3),
                (0, 12): lambda: emit_kt(1),
            }

            out_r = out_d.ap()
            pending_epi = [None]

            def flush_epi():
                if pending_epi[0] is not None:
                    pending_epi[0]()
                    pending_epi[0] = None

            for nh in range(NH):
                o_t = ps_o.tile([128, nw], F32, tag="o")
                d_t = ps_d.tile([128, nw], F32, tag="d")
                kslice = kT[:, nh * nw : (nh + 1) * nw]

                def emit_scores(p, nh=nh, kslice=kslice):
                    # both m-tiles of pair p, same 512 n-cols, one psum tile
                    s_t = ps_s.tile([128, 2, nw], F32, tag="s")
                    for j in range(2):
                        nc.tensor.matmul(
                            s_t[:, j, :], qT[:, ts(2 * p + j, 128)], kslice
                        )
                    return s_t

                s_tiles = {0: emit_scores(0), 1: emit_scores(1)}
                for p in range(PAIRS):
                    e_pr = expp.tile([128, 2, nw], F8, tag="e")
                    s_t = s_tiles.pop(p)
                    nc.scalar.activation(
                        out=e_pr.rearrange("p a b -> p (a b)"),
                        in_=s_t.rearrange("p a b -> p (a b)"),
                        func=mybir.ActivationFunctionType.Exp,
                        scale=SCALE,
                    )
                    # PE order: s(p+2), PV(p), den(p) -- one 1024-wide exp per
                    # pair feeds one DoubleRow PV and one DoubleRow den.
                    if p + 2 < PAIRS:
                        s_tiles[p + 2] = emit_scores(p + 2)
                    if p == 0:
                        # previous nh's epilogue. Emitted after this nh's first
                        # exp (so the ACT stream crosses the boundary without a
                        # gap) but BEFORE this nh's first PV/den write o_t/d_t:
                        # the hazard tracker follows emission order, so the
                        # epilogue's o_t/d_t reads must precede the overwrite.
                        flush_epi()
                    nc.tensor.matmul(
                        o_t,
                        vN[:, p, :, :],
                        e_pr,
                        start=p == 0,
                        stop=p == PAIRS - 1,
                        perf_mode=DR,
                        skip_group_check=True,
                    )
                    nc.tensor.matmul(
                        d_t,
                        ones8,
                        e_pr,
                        start=p == 0,
                        stop=p == PAIRS - 1,
                        perf_mode=DR,
                        skip_group_check=True,
                    )
                    staged.pop((nh, p), lambda: None)()

                def epilogue(nh=nh, o_t=o_t, d_t=d_t):
                    # out = o_t/den + bv, in [d, n] layout (bv per-partition)
                    rden = denp.tile([128, nw], F32, tag="rden")
                    o_f = outn.tile([128, 2, nw], F32R, tag="o_f")
                    nc.vector.reciprocal(out=rden, in_=d_t)
                    nc.vector.tensor_mul(out=o_f[:, 0, :], in0=o_t, in1=rden)
                    nc.vector.tensor_scalar_add(
                        out=o_f[:, 1, :], in0=o_f[:, 0, :], scalar1=bv_s
                    )
                    nc.sync.dma_start(
                        out=out_r[:, nh * nw : (nh + 1) * nw], in_=o_f[:, 1, :]
                    )

                pending_epi[0] = epilogue

            flush_epi()

        if unroll > 1:
            for _ in range(unroll):
                body()
        elif repeat > 1:
            with tc.For_i(0, repeat, 1):
                body()
        else:
            body()

    if split_waits:
        _split_excess_waits(nc)
    return nc


def make_in_maps(h_a, Wq, bq, Wk, bk, Wv, bv):
    """Per-core input dicts: permuted+transposed h plus stacked weights."""
    h_a = np.ascontiguousarray(h_a, dtype=np.float32)
    consts = {
        "wqkv": np.ascontiguousarray(np.stack([Wq, Wk, Wv]), np.float32),
        "bqkv": np.ascontiguousarray(np.stack([bq, bk, bv]), np.float32),
    }
    in_maps = []
    for core in range(NCORES):
        b, half = divmod(core, 2)
        n0 = half * CHUNK
        # chunk rows first, the rest after (order of the tail is irrelevant)
        perm = np.concatenate(
            [h_a[b, n0 : n0 + CHUNK], h_a[b, :n0], h_a[b, n0 + CHUNK :]], axis=0
        )
        in_maps.append({"h": np.ascontiguousarray(perm.T), **consts})
    return in_maps


_NC_CACHE = None
_LAST_RESULTS = None
TRACE = False
REPEAT = 1


def kernel(h_a, Wq, bq, Wk, bk, Wv, bv):
    global _NC_CACHE, _LAST_RESULTS
    if _NC_CACHE is None:
        _NC_CACHE = build_nc(repeat=REPEAT)
    nc = _NC_CACHE

    in_maps = make_in_maps(h_a, Wq, bq, Wk, bk, Wv, bv)
    res = run_bass_kernel_spmd(
        nc, in_maps, core_ids=list(range(NCORES)), trace=TRACE
    )
    _LAST_RESULTS = res

    out = np.empty((B, N, D), np.float32)
    for core in range(NCORES):
        b, half = divmod(core, 2)
        n0 = half * CHUNK
        out[b, n0 : n0 + CHUNK] = res.results[core]["out"].T
    return out


# revision 29
# speedup vs baseline: 1.2148x; 1.2127x over previous
"""Trainium2 Bass kernel for nn_AttentionLayer (B=4, N=4096, D=128).

Computation (per reference):
    Q = h @ Wq + bq ; K = h @ Wk + bk ; V = h @ Wv + bv          [B, N, 128]
    scores = einsum("bnd,bmd->bnm", K, Q) / sqrt(128)            [B, N, N]
    attn = softmax(scores, axis=-1)
    out = einsum("bnm,bmd->bnd", attn, V)                        [B, N, N->128]

Sharding: 8 cores = 4 batches x 2 chunks of 2048 K-rows (output rows).
Fully data-parallel SPMD - no collectives. Each core receives its batch's
h rows PERMUTED so that its own K-chunk rows come first (softmax/PV reduce
over the m index, which is order-independent), and TRANSPOSED to [128, 4096]
so it DMAs straight into the on-chip hT layout. The output is returned in
[128 d, 2048 n] layout and un-transposed on the host.

Per-core kernel (hT resident after one 2MB load):
  kT   = Wk^T-proj of hT[:, :2048] + bk  (f32r)   [128d, 2048n]
  qT   = Wq^T-proj of hT + bq            (f32r)   [128d, 4096m]
  vN   = hT_tile^T @ Wv  (direct natural V tiles, fp8e4, NO bias --
         softmax rows sum to 1 so  attn @ (hWv + bv) = attn @ (hWv) + bv,
         bv is added per-partition in the epilogue)  [4096m, 128d]
  per (nh quarter of n, pair p of m-tiles; one [128, 2, 512] psum score
  tile holds BOTH m-tiles of the pair for 512 n-cols):
      sT  = qT_mi^T @ kT_nh            (PSUM, f32r)       [128m, 2, 512n]
      e   = exp(sT / sqrt(128))        (one 1024-wide ACT -> fp8e4)
      oT += vN_pair^T @ e (DoubleRow)  (PSUM accum, fp8 pair contraction)
      den += ones8^T @ e  (DoubleRow)  (PSUM accum)
  out_nh = oT * (1/den) + bv           (DVE)  -> DMA out in [d, n] layout

ACT (exp) is the bottleneck engine (~66us busy; 64 x [128,1024] EXPs at
1 elem/lane/cycle @ 1.2 GHz -- the attention-math floor). PE ~53us,
DVE ~19us.  PSUM: 2x2-bank double-buffered score tiles + 2-bank
projection staging + 1-bank o/den accumulators = 8 banks.  Projection
groups beyond g0 are staged into the early main-loop pairs so the first
exp starts ~4us into the pass; each nh epilogue is deferred until after
the next nh's first exps (but before its first PV/den, which would
overwrite o/den) so the ACT exp stream never pauses at nh boundaries.
"""

import math
from contextlib import ExitStack

import numpy as np

import concourse.bass as bass
import concourse.mybir as mybir
import concourse.tile as tile
from concourse.bass_utils import run_bass_kernel_spmd
from concourse.tile import ScopedClock

F32 = mybir.dt.float32
F32R = mybir.dt.float32r
BF16 = mybir.dt.bfloat16
F8 = mybir.dt.float8e4
DR = mybir.MatmulPerfMode.DoubleRow

B, N, D = 4, 4096, 128
NCORES = 8
CHUNK = N * B // NCORES  # 2048 output rows per core
NW = 512  # n processed per PSUM-resident accumulation group
SCALE = 1.0 / math.sqrt(D)


def _patched_drain_and_barrier(self, tick_clock, wait_clock):
    # This walrus build rejects multiple sync waits on the Drain CTRL
    # instruction. Carry the waits on preceding SP nops (same engine =>
    # program order) and leave the drain nearly bare.
    nc = self.nc
    carrier = nc.sync.nop(nofuse=True, hint="drain_waits")
    wait_clock.add_sem_waits(carrier.ins, ScopedClock({None: tick_clock.global_clock}))
    si = carrier.ins.sync_info
    waits = list(si.on_wait) if si is not None else []
    if len(waits) > 1:
        by_name = {}
        for h in self.sems.allocated().values():
            by_name[getattr(h, "name", None) or str(h)] = h
        si.on_wait = [waits[0]]
        for w in waits[1:]:
            n = nc.sync.nop(nofuse=True, hint="drain_waits2")
            n.wait_op(by_name[w.ant_name], w.wait_value, "sem-ge")
    nc.sync.drain()
    nc.all_engine_barrier()
    assert self.sems is not None
    popped = nc._tile_sem_poison_stack.pop()
    assert popped is self._sem_poison
    nc.clear_and_free_semaphores(list(self.sems.allocated().values()))
    nc.all_engine_barrier()


def ts(i, sz):
    return slice(i * sz, (i + 1) * sz)


def _split_excess_waits(nc, maxw=1):
    # This walrus build allows at most ~1 sync wait per lowered instruction.
    # Hoist excess waits onto preceding same-engine NoOps.
    cnt = 0
    for f in nc.m.functions:
        for bb in f.blocks:
            out = []
            for inst in bb.instructions:
                si = inst.sync_info
                waits = list(si.on_wait) if si is not None else []
                if len(waits) > maxw:
                    for w in waits[: len(waits) - maxw]:
                        nop = mybir.InstNoOp(
                            name=f"{inst.name}-hw{cnt}",
                            engine=inst.engine,
                            ins=[],
                            outs=[],
                            sync_info=mybir.SyncInfo(on_wait=[w], on_update=[]),
                        )
                        out.append(nop)
                        cnt += 1
                    si.on_wait = waits[len(waits) - maxw :]
                out.append(inst)
            bb.instructions = out
    return cnt


def build_nc(n=N, chunk=CHUNK, nw=NW, split_waits=True, repeat=1, unroll=1):
    M_TILES = n // 128
    NH = chunk // nw  # 4
    GRP = 8  # m-tiles per projection/psum group
    GW = GRP * 128
    PAIRS = M_TILES // 2  # m-tile pairs per nh
    tile.TileContext._drain_and_barrier = _patched_drain_and_barrier
    nc = bass.Bass("TRN2", target_bir_lowering=False, debug=False, num_devices=NCORES)

    h_d = nc.dram_tensor("h", [D, n], F32R, kind="ExternalInput")  # pre-transposed
    w_d = nc.dram_tensor("wqkv", [3, D, D], F32, kind="ExternalInput")
    b_d = nc.dram_tensor("bqkv", [3, D], F32, kind="ExternalInput")
    out_d = nc.dram_tensor("out", [D, chunk], F32R, kind="ExternalOutput")

    with tile.TileContext(nc) as tc, ExitStack() as ctx:
        consts = ctx.enter_context(tc.tile_pool(name="consts", bufs=1))
        big = ctx.enter_context(tc.tile_pool(name="big", bufs=1))
        expp = ctx.enter_context(tc.tile_pool(name="expp", bufs=3))
        denp = ctx.enter_context(tc.tile_pool(name="denp", bufs=2))
        outn = ctx.enter_context(tc.tile_pool(name="outn", bufs=2))
        ps_s = ctx.enter_context(tc.tile_pool(name="ps_s", bufs=2, space="PSUM"))
        ps_p = ctx.enter_context(tc.tile_pool(name="ps_p", bufs=1, space="PSUM"))
        ps_o = ctx.enter_context(tc.tile_pool(name="ps_o", bufs=1, space="PSUM"))
        ps_d = ctx.enter_context(tc.tile_pool(name="ps_d", bufs=1, space="PSUM"))

        # ---- constants (3 DMAs + DVE casts; once, outside the repeat loop) --
        w_s = consts.tile([D, 3, D], F32, tag="w_s")
        b_s = consts.tile([D, 3], F32, tag="b_s")
        nc.sync.dma_start(out=w_s, in_=w_d.ap().rearrange("w c d -> c w d"))
        nc.sync.dma_start(out=b_s, in_=b_d.ap().rearrange("w d -> d w"))
        wq_r = consts.tile([D, D], F32R, tag="wq_r")
        wk_r = consts.tile([D, D], F32R, tag="wk_r")
        wv_r = consts.tile([D, D], F32R, tag="wv_r")
        nc.vector.tensor_copy(out=wq_r, in_=w_s[:, 0, :])
        nc.vector.tensor_copy(out=wk_r, in_=w_s[:, 1, :])
        nc.vector.tensor_copy(out=wv_r, in_=w_s[:, 2, :])
        bq_s, bk_s, bv_s = b_s[:, 0:1], b_s[:, 1:2], b_s[:, 2:3]
        ones_f = consts.tile([128, 256], F32, tag="ones_f")
        ones8 = consts.tile([128, 2, 128], F8, tag="ones8")
        nc.vector.memset(ones_f, 1.0)
        nc.vector.tensor_copy(out=ones8, in_=ones_f.rearrange("p (a b) -> p a b", a=2))

        # ---- load hT (once; outside the repeat loop) as 4 independent
        # tiles so the first projections start after ~1/4 of the transfer --
        hTt = [
            big.tile([128, GW], F32R, name=f"hT{g}", tag=f"hT{g}")
            for g in range(M_TILES // GRP)
        ]
        for g, t in enumerate(hTt):
            nc.sync.dma_start(out=t, in_=h_d.ap()[:, ts(g, GW)])

        qT = big.tile([128, n], F32R, tag="qT")
        kT = big.tile([128, chunk], F32R, tag="kT")
        vN = big.tile([128, M_TILES // 2, 2, 128], F8, tag="vN")

        def emit_kt(g):
            p_t = ps_p.tile([128, GW], F32, tag="pp")
            for j in range(GW // 512):
                nc.tensor.matmul(p_t[:, ts(j, 512)], wk_r, hTt[g][:, ts(j, 512)])
            nc.vector.tensor_scalar_add(
                out=kT[:, ts(g, GW)], in0=p_t[:, :GW], scalar1=bk_s
            )

        def emit_qt(g):
            p_t = ps_p.tile([128, GW], F32, tag="pp")
            for j in range(GW // 512):
                nc.tensor.matmul(p_t[:, ts(j, 512)], wq_r, hTt[g][:, ts(j, 512)])
            nc.vector.tensor_scalar_add(
                out=qT[:, ts(g, GW)], in0=p_t[:, :GW], scalar1=bq_s
            )

        def emit_vn(g):
            t_ps = ps_p.tile([128, GW], F32, tag="pp")
            for k in range(GRP):
                nc.tensor.matmul(t_ps[:, ts(k, 128)], hTt[g][:, ts(k, 128)], wv_r)
            nc.vector.tensor_copy(
                out=vN[:, ts(g, GRP // 2), :, :],
                in_=t_ps.rearrange("p (t j d) -> p t j d", j=2, d=128),
            )

        def body():
            # head: only group 0 of each projection before the main loop;
            # the rest is staged into the early pairs below.
            emit_kt(0)
            emit_qt(0)
            emit_vn(0)

            # staged projection work, keyed by (nh, pair): emitted into the
            # PE stream right after that pair's matmuls. Group g of qT/vN is
            # consumed starting at pair 4*g of EVERY nh (each nh sweeps all
            # m-tiles), so everything beyond g0 must land within nh 0.
            staged = {
                (0, 0): lambda: emit_qt(1),
                (0, 2): lambda: emit_vn(1),
                (0, 4): lambda: emit_qt(2),
                (0, 6): lambda: emit_vn(2),
                (0, 8): lambda: emit_qt(3),
                (0, 10): lambda: emit_vn(3),
                (0, 12): lambda: emit_kt(1),
            }

            out_r = out_d.ap()
            pending_epi = [None]

            def flush_epi():
                if pending_epi[0] is not None:
                    pending_epi[0]()
                    pending_epi[0] = None

            for nh in range(NH):
                o_t = ps_o.tile([128, nw], F32, tag="o")
                d_t = ps_d.tile([128, nw], F32, tag="d")
                kslice = kT[:, nh * nw : (nh + 1) * nw]

                def emit_scores(p, nh=nh, kslice=kslice):
                    # both m-tiles of pair p, same 512 n-cols, one psum tile
                    s_t = ps_s.tile([128, 2, nw], F32, tag="s")
                    for j in range(2):
                        nc.tensor.matmul(
                            s_t[:, j, :], qT[:, ts(2 * p + j, 128)], kslice
                        )
                    return s_t

                s_tiles = {0: emit_scores(0), 1: emit_scores(1)}
                for p in range(PAIRS):
                    e_pr = expp.tile([128, 2, nw], F8, tag="e")
                    s_t = s_tiles.pop(p)
                    nc.scalar.activation(
                        out=e_pr.rearrange("p a b -> p (a b)"),
                        in_=s_t.rearrange("p a b -> p (a b)"),
                        func=mybir.ActivationFunctionType.Exp,
                        scale=SCALE,
                    )
                    # PE order: s(p+2), PV(p), den(p) -- one 1024-wide exp per
                    # pair feeds one DoubleRow PV and one DoubleRow den.
                    if p + 2 < PAIRS:
                        s_tiles[p + 2] = emit_scores(p + 2)
                    if p == 0:
                        # previous nh's epilogue. Emitted after this nh's first
                        # exp (so the ACT stream crosses the boundary without a
                        # gap) but BEFORE this nh's first PV/den write o_t/d_t:
                        # the hazard tracker follows emission order, so the
                        # epilogue's o_t/d_t reads must precede the overwrite.
                        flush_epi()
                    nc.tensor.matmul(
                        o_t,
                        vN[:, p, :, :],
                        e_pr,
                        start=p == 0,
                        stop=p == PAIRS - 1,
                        perf_mode=DR,
                        skip_group_check=True,
                    )
                    nc.tensor.matmul(
                        d_t,
                        ones8,
                        e_pr,
                        start=p == 0,
                        stop=p == PAIRS - 1,
                        perf_mode=DR,
                        skip_group_check=True,
                    )
                    staged.pop((nh, p), lambda: None)()

                def epilogue(nh=nh, o_t=o_t, d_t=d_t):
                    # out = o_t/den + bv, in [d, n] layout (bv per-partition)
                    rden = denp.tile([128, nw], F32, tag="rden")
                    o_f = outn.tile([128, 2, nw], F32R, tag="o_f")
                    nc.vector.reciprocal(out=rden, in_=d_t)
                    nc.vector.tensor_mul(out=o_f[:, 0, :], in0=o_t, in1=rden)
                    nc.vector.tensor_scalar_add(
                        out=o_f[:, 1, :], in0=o_f[:, 0, :], scalar1=bv_s
                    )
                    nc.sync.dma_start(
                        out=out_r[:, nh * nw : (nh + 1) * nw], in_=o_f[:, 1, :]
                    )

                pending_epi[0] = epilogue

            flush_epi()

        if unroll > 1:
            for _ in range(unroll):
                body()
        elif repeat > 1:
            with tc.For_i(0, repeat, 1):
                body()
        else:
            body()

    if split_waits:
        _split_excess_waits(nc)
    return nc


def make_in_maps(h_a, Wq, bq, Wk, bk, Wv, bv):
    """Per-core input dicts: permuted+transposed h plus stacked weights."""
    h_a = np.ascontiguousarray(h_a, dtype=np.float32)
    consts = {
        "wqkv": np.ascontiguousarray(np.stack([Wq, Wk, Wv]), np.float32),
        "bqkv": np.ascontiguousarray(np.stack([bq, bk, bv]), np.float32),
    }
    in_maps = []
    for core in range(NCORES):
        b, half = divmod(core, 2)
        n0 = half * CHUNK
        # chunk rows first, the rest after (order of the tail is irrelevant)
        perm = np.concatenate(
            [h_a[b, n0 : n0 + CHUNK], h_a[b, :n0], h_a[b, n0 + CHUNK :]], axis=0
        )
        in_maps.append({"h": np.ascontiguousarray(perm.T), **consts})
    return in_maps


_NC_CACHE = None
_LAST_RESULTS = None
TRACE = False
REPEAT = 1


def kernel(h_a, Wq, bq, Wk, bk, Wv, bv):
    global _NC_CACHE, _LAST_RESULTS
    if _NC_CACHE is None:
        _NC_CACHE = build_nc(repeat=REPEAT)
    nc = _NC_CACHE

    in_maps = make_in_maps(h_a, Wq, bq, Wk, bk, Wv, bv)
    res = run_bass_kernel_spmd(
        nc, in_maps, core_ids=list(range(NCORES)), trace=TRACE
    )
    _LAST_RESULTS = res

    out = np.empty((B, N, D), np.float32)
    for core in range(NCORES):
        b, half = divmod(core, 2)
        n0 = half * CHUNK
        out[b, n0 : n0 + CHUNK] = res.results[core]["out"].T
    return out
